# revision 20
# baseline (speedup 1.0000x reference)
"""Trainium2 Bass kernel: DeformableValueAttention.

Full-input contract: kernel(**inputs) takes the unsharded inputs of
reference.setup_inputs() and returns the full [B, C, H, W] output.

Sharding: 8 cores = (batch b, query-half qh). Each core computes attention
for 512 queries x all 1024 keys of one batch and produces a disjoint
[C, 512] column-slice of the output -- no cross-core reduction.

Per-core algorithm (all layouts channels-on-partitions = the natural [C, N]
memory layout of the problem):
  QT = (Wq/8) @ xq          [C, Nq]   (1/8 attention scale folded into Wq)
  KT = Wk @ xkv             [C, N]
  V  = xkv^T @ Wv^T         [N, C]    (keys on partitions)
  Vd = G^T.T @ V            [N, C]    grid_sample as a banded sparse matmul;
                                      the (1+gamma*sal) value modulation is
                                      folded into G's weights on the host
  per head h (2-head PE row-group packing, K=64):
    S^T = KT_h^T @ QT_h     [N, Nq]   scores, queries on free dim
    Pu  = exp(S^T)          bf16      no max-subtraction needed: |scores|<~6
    O^T = [Vd_h | 1]^T @ Pu [65, Nq]  ones-column yields softmax denominators
                                      in row 64 for free
    O   = O^T[0:64] * bcast(1/denom)  (GPSIMD partition_broadcast)
  out^T = Wo @ O^T          [C, Nq]   == the output's [C, N] layout directly

Phase order is chosen so the 32 exp activations (the ACT engine's ~37us
stream) start as soon as QT/KT are done and overlap the V/Vd/O matmuls.
Input DMAs are batched into one wide transfer per tensor and spread across
four engine queues in consumption order.

Notes on fidelity vs reference.py:
  - P_thermal adds a per-query constant to scores pre-softmax; softmax is
    exactly invariant to that, so it is skipped (fp-rounding-level diff).
  - All biases in setup_inputs() are zeros; if a caller ever passes nonzero
    biases we fall back to a numpy reference implementation.
  - Matmuls run in bf16 (fp32 PSUM accumulation); measured end-to-end
    L2 relative error ~5e-3 vs the fp32 reference.
"""

import os
import sys

import numpy as np
import ml_dtypes

try:
    import concourse.bass as bass
except ImportError:  # pragma: no cover - path fallback for bare containers
    sys.path.insert(0, "/opt/trn_rl_repo")
    import concourse.bass as bass

import concourse.bacc as bacc
import concourse.tile as tile
from concourse import mybir
from concourse.bass_utils import run_bass_kernel_spmd

B, C, HH, WW = 4, 512, 32, 32
N = HH * WW          # 1024 spatial positions (= keys)
NQ = N // 2          # queries per core
NH, HD = 8, 64       # heads, head dim
P = 128
CT = C // P          # 4 channel partition-tiles
NKT = N // P         # 8 key tiles
NCORES = 8
BF16 = mybir.dt.bfloat16
FP32 = mybir.dt.float32
NP_BF16 = ml_dtypes.bfloat16


# --------------------------------------------------------------------------
# host-side helpers
# --------------------------------------------------------------------------

def _gather_T(offsets_b, salf_b):
    """GT[k, n]: weight of source pixel k in grid-sampled output pixel n,
    with the per-source value modulation salf folded in. fp32 [N, N]."""
    ys = np.linspace(-1.0, 1.0, HH)
    xs = np.linspace(-1.0, 1.0, WW)
    gy, gx = np.meshgrid(ys, xs, indexing="ij")
    x = ((gx + offsets_b[0] / (WW / 2.0) + 1.0) * WW - 1.0) * 0.5
    y = ((gy + offsets_b[1] / (HH / 2.0) + 1.0) * HH - 1.0) * 0.5
    x = np.clip(x, 0.0, WW - 1.0)
    y = np.clip(y, 0.0, HH - 1.0)
    x0 = np.floor(x); y0 = np.floor(y)
    wx = x - x0; wy = y - y0
    x0i = x0.astype(np.int64); y0i = y0.astype(np.int64)
    x1i = np.minimum(x0i + 1, WW - 1); y1i = np.minimum(y0i + 1, HH - 1)
    GT = np.zeros((N, N), np.float32)
    n_idx = np.arange(N)
    for yi, xi, w in ((y0i, x0i, (1 - wx) * (1 - wy)),
                      (y0i, x1i, wx * (1 - wy)),
                      (y1i, x0i, (1 - wx) * wy),
                      (y1i, x1i, wx * wy)):
        np.add.at(GT, ((yi * WW + xi).reshape(-1), n_idx),
                  w.reshape(-1).astype(np.float32))
    GT *= salf_b[:, None]
    return GT


def _reference_numpy(q_feat, kv_feat, offsets, saliency_map, P_thermal,
                     Wq, bq, Wk, bk, Wv, bv, Wo, bo, lambda_p, gamma_val):
    """Plain numpy port of reference.py -- correctness fallback only."""
    qf = q_feat.reshape(B, C, N).transpose(0, 2, 1)
    kf = kv_feat.reshape(B, C, N).transpose(0, 2, 1)

    def heads(x, Wm, bm):
        return (x @ Wm.T + bm).reshape(B, N, NH, HD).transpose(0, 2, 1, 3)

    Q = heads(qf, Wq, bq)
    K = heads(kf, Wk, bk)
    V = heads(kf, Wv, bv)
    attn = np.einsum("bhqd,bhkd->bhqk", Q, K) * (HD ** -0.5)
    attn = attn + float(lambda_p) * P_thermal.reshape(B, 1, N, 1)
    attn = attn - attn.max(axis=-1, keepdims=True)
    w = np.exp(attn)
    w /= w.sum(axis=-1, keepdims=True)
    Vm = V * (1.0 + float(gamma_val) * saliency_map.reshape(B, 1, N, 1))
    Vsp = Vm.transpose(0, 2, 1, 3).reshape(B, N, C).transpose(0, 2, 1)
    Vd = np.empty_like(Vsp)
    for b in range(B):
        GT = _gather_T(offsets[b], np.ones(N, np.float32))
        Vd[b] = Vsp[b] @ GT
    Vdf = Vd.reshape(B, C, N).transpose(0, 2, 1).reshape(B, N, NH, HD).transpose(0, 2, 1, 3)
    out = np.einsum("bhqk,bhkd->bhqd", w, Vdf)
    out = out.transpose(0, 2, 1, 3).reshape(B, N, C)
    out = out @ Wo.T + bo
    return out.transpose(0, 2, 1).reshape(B, C, HH, WW).astype(np.float32)


# --------------------------------------------------------------------------
# device program
# --------------------------------------------------------------------------

def _build_program(chunks):
    """chunks: ordered list of (m, k) gather-tile pairs; same for all cores."""
    nch = len(chunks)
    chunks_for_m = {m: [] for m in range(NKT)}
    for idx, (m, k) in enumerate(chunks):
        chunks_for_m[m].append((idx, k))

    nc = bacc.Bacc(None, target_bir_lowering=False, debug=False)
    xq_d = nc.declare_dram_parameter("xq", [C, NQ], BF16, isOutput=False)
    xkv_d = nc.declare_dram_parameter("xkv", [C, N], BF16, isOutput=False)
    wq_d = nc.declare_dram_parameter("wqT", [C, C], BF16, isOutput=False)
    wk_d = nc.declare_dram_parameter("wkT", [C, C], BF16, isOutput=False)
    wv_d = nc.declare_dram_parameter("wvT", [C, C], BF16, isOutput=False)
    wo_d = nc.declare_dram_parameter("woT", [C, C], BF16, isOutput=False)
    gt_d = nc.declare_dram_parameter("gt", [nch, P, P], BF16, isOutput=False)
    out_d = nc.declare_dram_parameter("outT", [C, NQ], FP32, isOutput=True)

    with tile.TileContext(nc) as tc:
        with tc.tile_pool(name="const", bufs=1) as const, \
             tc.tile_pool(name="work", bufs=1) as work, \
             tc.tile_pool(name="pu_pool", bufs=1) as pu_pool, \
             tc.tile_pool(name="sm", bufs=4) as sm, \
             tc.tile_pool(name="psp", bufs=2, space="PSUM") as psp:

            # ---- per-tile contiguous loads (each DMA = 128 consecutive DRAM
            # rows), spread across the three DMA-capable queues in
            # consumption order: QT needs wq+xq first, then KT needs wk+xkv,
            # then wv / gt / wo.
            def load_rows(engine, dram, nm, nrows_tiles, width, rows_per_tile=P):
                ts = []
                for t in range(nrows_tiles):
                    tl = const.tile([rows_per_tile, width], BF16,
                                    name=f"{nm}{t}", tag=f"{nm}{t}")
                    engine.dma_start(
                        out=tl[:],
                        in_=dram[t * rows_per_tile:(t + 1) * rows_per_tile, :])
                    ts.append(tl)
                return ts

            wq_sb = load_rows(nc.sync, wq_d, "wq", CT, C)
            xq_sb = load_rows(nc.scalar, xq_d, "xq", CT, NQ)
            xkv_sb = load_rows(nc.gpsimd, xkv_d, "xkv", CT, N)
            wk_sb = load_rows(nc.gpsimd, wk_d, "wk", CT, C)
            wv_sb = load_rows(nc.sync, wv_d, "wv", CT, C)
            # gather-matrix chunks and the head-major Wo arrive late in the
            # schedule; single wide DMAs on otherwise-idle queues are fine.
            gt_w = const.tile([P, nch * P], BF16, name="gtw", tag="gtw")
            nc.gpsimd.dma_start(
                out=gt_w[:].rearrange("p (c j) -> p c j", j=P),
                in_=gt_d[:].rearrange("c p j -> p c j"))
            wo_w = const.tile([HD, NH * C], BF16, name="wow", tag="wow")
            nc.sync.dma_start(
                out=wo_w[:].rearrange("p (h n) -> p h n", n=C),
                in_=wo_d[:].rearrange("(h p) n -> p h n", p=HD))

            def wq_s(k, t):
                return wq_sb[k][:, t * P:(t + 1) * P]

            def wk_s(k, t):
                return wk_sb[k][:, t * P:(t + 1) * P]

            def xkv_s(k, lo, width):
                return xkv_sb[k][:, lo: lo + width]

            # ---- emission helpers -----------------------------------------
            qt_sb, kt_sb, v_sb, vd_sb = {}, {}, {}, {}
            pu_tiles, ps_o_tiles = {}, {}
            o_sb = [None] * NH

            def emit_qt(t):
                ps = psp.tile([P, NQ], FP32, name=f"psq{t}", tag="ps_proj", bufs=2)
                for k in range(CT):
                    nc.tensor.matmul(ps[:], lhsT=wq_s(k, t), rhs=xq_sb[k][:],
                                     start=(k == 0), stop=(k == CT - 1))
                qt = work.tile([P, NQ], BF16, name=f"qt{t}", tag=f"qt{t}")
                nc.vector.tensor_copy(qt[:], ps[:])
                qt_sb[t] = qt

            def emit_kt(t):
                kt = work.tile([P, N], BF16, name=f"kt{t}", tag=f"kt{t}")
                for half in range(2):
                    ps = psp.tile([P, NQ], FP32, name=f"psk{t}_{half}",
                                  tag="ps_proj", bufs=2)
                    for k in range(CT):
                        nc.tensor.matmul(ps[:], lhsT=wk_s(k, t),
                                         rhs=xkv_s(k, half * NQ, NQ),
                                         start=(k == 0), stop=(k == CT - 1))
                    nc.vector.tensor_copy(kt[:, half * NQ:(half + 1) * NQ], ps[:])
                kt_sb[t] = kt

            def emit_s_chunk(t, m):
                ps_s = psp.tile([P, 2 * NQ], FP32, name=f"pss{t}_{m}",
                                tag="ps_s", bufs=2)
                # head A on PE rows 0-63, head B on rows 64-127: the two
                # K=64 matmuls occupy disjoint row-groups and overlap.
                kt, qt = kt_sb[t], qt_sb[t]
                nc.tensor.matmul(ps_s[:, 0:NQ],
                                 lhsT=kt[0:HD, m * P:(m + 1) * P],
                                 rhs=qt[0:HD, :], start=True, stop=True)
                nc.tensor.matmul(ps_s[:, NQ:2 * NQ],
                                 lhsT=kt[HD:P, m * P:(m + 1) * P],
                                 rhs=qt[HD:P, :], start=True, stop=True)
                pu = pu_pool.tile([P, 2 * NQ], BF16, name=f"pu{t}_{m}",
                                  tag=f"pu{t}_{m}")
                nc.scalar.activation(out=pu[:], in_=ps_s[:],
                                     func=mybir.ActivationFunctionType.Exp)
                pu_tiles[(t, m)] = pu

            def emit_v(m):
                ps = psp.tile([P, C], FP32, name=f"psv{m}", tag="ps_proj", bufs=2)
                for k in range(CT):
                    nc.tensor.matmul(ps[:], lhsT=xkv_s(k, m * P, P),
                                     rhs=wv_sb[k][:],
                                     start=(k == 0), stop=(k == CT - 1))
                tl = work.tile([P, C], BF16, name=f"v{m}", tag=f"v{m}")
                nc.vector.tensor_copy(tl[:], ps[:])
                v_sb[m] = tl

            def emit_vd(m):
                # grid-sample as banded matmul; vd layout per key-tile:
                # [P, 8*65]; cols h*65..h*65+63 = head h channels,
                # col h*65+64 = 1.0 (softmax denominator trick).
                ps = psp.tile([P, C], FP32, name=f"psvd{m}", tag="ps_proj", bufs=2)
                lst = chunks_for_m[m]
                for j, (idx, k) in enumerate(lst):
                    nc.tensor.matmul(ps[:], lhsT=gt_w[:, idx * P:(idx + 1) * P],
                                     rhs=v_sb[k][:],
                                     start=(j == 0), stop=(j == len(lst) - 1))
                tl = work.tile([P, NH * (HD + 1)], BF16,
                               name=f"vd{m}", tag=f"vd{m}")
                tl3 = tl[:].rearrange("p (h e) -> p h e", e=HD + 1)
                nc.vector.tensor_copy(
                    tl3[:, :, 0:HD],
                    ps[:].rearrange("p (h e) -> p h e", e=HD))
                nc.vector.memset(tl3[:, :, HD:HD + 1], 1.0)
                vd_sb[m] = tl

            def emit_o_chunk(hp, m):
                if hp not in ps_o_tiles:
                    ps_o_tiles[hp] = (
                        psp.tile([P, NQ], FP32, name=f"pso{hp}a", tag="ps_o",
                                 bufs=2),
                        psp.tile([P, NQ], FP32, name=f"pso{hp}b", tag="ps_o",
                                 bufs=2))
                ps_oA, ps_oB = ps_o_tiles[hp]
                hA, hB = 2 * hp, 2 * hp + 1
                pu = pu_tiles[(hp, m)]
                nc.tensor.matmul(
                    ps_oA[0:HD + 1, :],
                    lhsT=vd_sb[m][:, hA * (HD + 1):(hA + 1) * (HD + 1)],
                    rhs=pu[:, 0:NQ],
                    start=(m == 0), stop=(m == NKT - 1))
                nc.tensor.matmul(
                    ps_oB[0:HD + 1, :],
                    lhsT=vd_sb[m][:, hB * (HD + 1):(hB + 1) * (HD + 1)],
                    rhs=pu[:, NQ:2 * NQ],
                    start=(m == 0), stop=(m == NKT - 1))

            def emit_norm(hp):
                # normalize: O_h = O_u[0:64] * (1/denom), denom = row 64.
                # Evacuate PSUM via ACT copy (~0.7us) so the ps_o slot frees
                # fast; the lane-starved [1, NQ] DVE reciprocal overlaps the
                # next pair's matmuls. (ACT exp(-ln) would be faster per-op
                # but ping-pongs the activation LUT set against the softmax
                # Exps -- measured 2.5us per switch.)
                ps_oA, ps_oB = ps_o_tiles[hp]
                for h, ps_o in ((2 * hp, ps_oA), (2 * hp + 1, ps_oB)):
                    ou = sm.tile([HD + 1, NQ], FP32, name=f"ou{h}", tag="ou",
                                 bufs=2)
                    nc.scalar.copy(ou[:], ps_o[0:HD + 1, :])
                    rec = sm.tile([1, NQ], FP32, name=f"rec{h}", tag="rec",
                                  bufs=4)
                    nc.vector.reciprocal(rec[:], ou[HD:HD + 1, :])
                    bc = sm.tile([HD, NQ], FP32, name=f"bc{h}", tag="bc", bufs=2)
                    nc.gpsimd.partition_broadcast(bc[:], rec[:])
                    ot = work.tile([HD, NQ], BF16, name=f"o{h}", tag=f"o{h}")
                    nc.vector.tensor_mul(ot[:], ou[0:HD, :], bc[:])
                    o_sb[h] = ot

            # ---- interleaved emission schedule ----------------------------
            # The 32 exp activations (1.15us each) pace the S blocks; between
            # consecutive S chunks we emit one unit of independent PE work
            # (later projections, V, Vd, pair-0 O chunks) so the PE never
            # stalls on a full ps_s pool.
            fillers = ([lambda t=t: (emit_qt(t), emit_kt(t)) for t in (1, 2, 3)]
                       + [lambda m=m: emit_v(m) for m in range(NKT)]
                       + [lambda m=m: emit_vd(m) for m in range(NKT)]
                       + [lambda m=m: emit_o_chunk(0, m) for m in range(NKT)])
            fi = 0
            emit_qt(0)
            emit_kt(0)
            for t in range(CT):
                for m in range(NKT):
                    emit_s_chunk(t, m)
                    if fi < len(fillers):
                        fillers[fi]()
                        fi += 1
            while fi < len(fillers):
                fillers[fi]()
                fi += 1

            emit_norm(0)
            for hp in range(1, CT):
                for m in range(NKT):
                    emit_o_chunk(hp, m)
                emit_norm(hp)

            # ---- out^T = Wo @ O^T : [C, NQ] fp32 --------------------------
            # h-outer accumulation in two halves: heads 0-5 accumulate while
            # the last head pair is still normalizing, leaving only the h=6,7
            # matmuls in the serial tail. PSUM slots reuse the ps_proj tag
            # (its last user, the Vd accumulators, is long done by now).
            for whalf in range(2):
                tpair = (2 * whalf, 2 * whalf + 1)
                ps_w = {}
                for t in tpair:
                    ps_w[t] = psp.tile([P, NQ], FP32, name=f"psw{t}",
                                       tag="ps_proj", bufs=2)
                for h in range(NH):
                    for t in tpair:
                        nc.tensor.matmul(
                            ps_w[t][:],
                            lhsT=wo_w[:, h * C + t * P: h * C + (t + 1) * P],
                            rhs=o_sb[h][:],
                            start=(h == 0), stop=(h == NH - 1))
                for t in tpair:
                    ob = sm.tile([P, NQ], FP32, name=f"ob{t}", tag="ob", bufs=2)
                    nc.vector.tensor_copy(ob[:], ps_w[t][:])
                    nc.sync.dma_start(out=out_d[t * P:(t + 1) * P, :], in_=ob[:])

    nc.compile()
    return nc


# --------------------------------------------------------------------------
# public entry points
# --------------------------------------------------------------------------

def _prepare(inputs):
    q = np.ascontiguousarray(inputs["q_feat"], np.float32).reshape(B, C, N)
    kv = np.ascontiguousarray(inputs["kv_feat"], np.float32).reshape(B, C, N)
    offsets = np.asarray(inputs["offsets"], np.float32)
    sal = np.asarray(inputs["saliency_map"], np.float32).reshape(B, N)
    gamma = float(np.asarray(inputs["gamma_val"]))

    GTs = [_gather_T(offsets[b], 1.0 + gamma * sal[b]) for b in range(B)]

    # union band-sparsity pattern of the gather matmul across batches, so the
    # SPMD program is identical on every core
    chunks = []
    for m in range(NKT):
        for k in range(NKT):
            if any(GTs[b][k * P:(k + 1) * P, m * P:(m + 1) * P].any()
                   for b in range(B)):
                chunks.append((m, k))

    wqT = np.ascontiguousarray((np.asarray(inputs["Wq"], np.float32).T
                                * (HD ** -0.5)).astype(NP_BF16))
    wkT = np.ascontiguousarray(np.asarray(inputs["Wk"], np.float32).T.astype(NP_BF16))
    wvT = np.ascontiguousarray(np.asarray(inputs["Wv"], np.float32).T.astype(NP_BF16))
    woT = np.ascontiguousarray(np.asarray(inputs["Wo"], np.float32).T.astype(NP_BF16))

    in_maps = []
    for core in range(NCORES):
        b, qh = core // 2, core % 2
        gt_stack = np.stack([
            np.ascontiguousarray(
                GTs[b][k * P:(k + 1) * P, m * P:(m + 1) * P]).astype(NP_BF16)
            for (m, k) in chunks])
        in_maps.append({
            "xq": np.ascontiguousarray(
                q[b][:, qh * NQ:(qh + 1) * NQ]).astype(NP_BF16),
            "xkv": np.ascontiguousarray(kv[b]).astype(NP_BF16),
            "wqT": wqT, "wkT": wkT, "wvT": wvT, "woT": woT,
            "gt": gt_stack,
        })

    def assemble(results):
        out = np.empty((B, C, N), np.float32)
        for core in range(NCORES):
            b, qh = core // 2, core % 2
            out[b][:, qh * NQ:(qh + 1) * NQ] = results[core]["outT"]
        return out.reshape(B, C, HH, WW)

    nc = _build_program(chunks)
    return nc, in_maps, assemble


def _needs_fallback(inputs):
    try:
        if tuple(np.shape(inputs["q_feat"])) != (B, C, HH, WW):
            return True
        for bias in ("bq", "bk", "bv", "bo"):
            if np.any(np.asarray(inputs[bias], np.float32) != 0.0):
                return True
    except Exception:
        return True
    return False


def kernel(**inputs) -> np.ndarray:
    if _needs_fallback(inputs):
        return _reference_numpy(**{k: np.asarray(v, np.float32)
                                   for k, v in inputs.items()})
    nc, in_maps, assemble = _prepare(inputs)
    res = run_bass_kernel_spmd(nc, in_maps, core_ids=list(range(NCORES)))
    return assemble(res.results)


def kernel_traced(trace_cores=(0,), **inputs):
    """Like kernel() but returns (output, exec_time_ns, trace_path)."""
    nc, in_maps, assemble = _prepare(inputs)
    res = run_bass_kernel_spmd(nc, in_maps, core_ids=list(range(NCORES)),
                               trace=True, trace_cores=list(trace_cores))
    trace_path = None
    if res.instructions_and_trace is not None:
        trace_path = res.instructions_and_trace[1]
    return assemble(res.results), res.exec_time_ns, trace_path


# revision 22
# speedup vs baseline: 1.1035x; 1.1035x over previous
"""Trainium2 Bass kernel: DeformableValueAttention.

Full-input contract: kernel(**inputs) takes the unsharded inputs of
reference.setup_inputs() and returns the full [B, C, H, W] output.

Sharding: 8 cores = (batch b, query-half qh). Each core computes attention
for 512 queries x all 1024 keys of one batch and produces a disjoint
[C, 512] column-slice of the output -- no cross-core reduction.

Per-core algorithm (all layouts channels-on-partitions = the natural [C, N]
memory layout of the problem):
  QT = (Wq/8) @ xq          [C, Nq]   (1/8 attention scale folded into Wq)
  KT = Wk @ xkv             [C, N]
  V  = xkv^T @ Wv^T         [N, C]    (keys on partitions)
  Vd = G^T.T @ V            [N, C]    grid_sample as a banded sparse matmul;
                                      the (1+gamma*sal) value modulation is
                                      folded into G's weights on the host
  per head h (2-head PE row-group packing, K=64):
    S^T = KT_h^T @ QT_h     [N, Nq]   scores, queries on free dim
    Pu  = exp(S^T)          bf16      no max-subtraction needed: |scores|<~6
    O^T = [Vd_h | 1]^T @ Pu [65, Nq]  ones-column yields softmax denominators
                                      in row 64 for free
    O   = O^T[0:64] * bcast(1/denom)  (GPSIMD partition_broadcast)
  out^T = Wo @ O^T          [C, Nq]   == the output's [C, N] layout directly

Phase order is chosen so the 32 exp activations (the ACT engine's ~37us
stream) start as soon as QT/KT are done and overlap the V/Vd/O matmuls.
Input DMAs are batched into one wide transfer per tensor and spread across
four engine queues in consumption order.

Notes on fidelity vs reference.py:
  - P_thermal adds a per-query constant to scores pre-softmax; softmax is
    exactly invariant to that, so it is skipped (fp-rounding-level diff).
  - All biases in setup_inputs() are zeros; if a caller ever passes nonzero
    biases we fall back to a numpy reference implementation.
  - Matmuls run in bf16 (fp32 PSUM accumulation); measured end-to-end
    L2 relative error ~5e-3 vs the fp32 reference.
"""

import os
import sys

import numpy as np
import ml_dtypes

try:
    import concourse.bass as bass
except ImportError:  # pragma: no cover - path fallback for bare containers
    sys.path.insert(0, "/opt/trn_rl_repo")
    import concourse.bass as bass

import concourse.bacc as bacc
import concourse.tile as tile
from concourse import mybir
from concourse.bass_utils import run_bass_kernel_spmd

B, C, HH, WW = 4, 512, 32, 32
N = HH * WW          # 1024 spatial positions (= keys)
NQ = N // 2          # queries per core
NH, HD = 8, 64       # heads, head dim
P = 128
CT = C // P          # 4 channel partition-tiles
NKT = N // P         # 8 key tiles
NCORES = 8
BF16 = mybir.dt.bfloat16
FP32 = mybir.dt.float32
NP_BF16 = ml_dtypes.bfloat16


# --------------------------------------------------------------------------
# host-side helpers
# --------------------------------------------------------------------------

def _gather_T(offsets_b, salf_b):
    """GT[k, n]: weight of source pixel k in grid-sampled output pixel n,
    with the per-source value modulation salf folded in. fp32 [N, N]."""
    ys = np.linspace(-1.0, 1.0, HH)
    xs = np.linspace(-1.0, 1.0, WW)
    gy, gx = np.meshgrid(ys, xs, indexing="ij")
    x = ((gx + offsets_b[0] / (WW / 2.0) + 1.0) * WW - 1.0) * 0.5
    y = ((gy + offsets_b[1] / (HH / 2.0) + 1.0) * HH - 1.0) * 0.5
    x = np.clip(x, 0.0, WW - 1.0)
    y = np.clip(y, 0.0, HH - 1.0)
    x0 = np.floor(x); y0 = np.floor(y)
    wx = x - x0; wy = y - y0
    x0i = x0.astype(np.int64); y0i = y0.astype(np.int64)
    x1i = np.minimum(x0i + 1, WW - 1); y1i = np.minimum(y0i + 1, HH - 1)
    GT = np.zeros((N, N), np.float32)
    n_idx = np.arange(N)
    for yi, xi, w in ((y0i, x0i, (1 - wx) * (1 - wy)),
                      (y0i, x1i, wx * (1 - wy)),
                      (y1i, x0i, (1 - wx) * wy),
                      (y1i, x1i, wx * wy)):
        np.add.at(GT, ((yi * WW + xi).reshape(-1), n_idx),
                  w.reshape(-1).astype(np.float32))
    GT *= salf_b[:, None]
    return GT


def _reference_numpy(q_feat, kv_feat, offsets, saliency_map, P_thermal,
                     Wq, bq, Wk, bk, Wv, bv, Wo, bo, lambda_p, gamma_val):
    """Plain numpy port of reference.py -- correctness fallback only."""
    qf = q_feat.reshape(B, C, N).transpose(0, 2, 1)
    kf = kv_feat.reshape(B, C, N).transpose(0, 2, 1)

    def heads(x, Wm, bm):
        return (x @ Wm.T + bm).reshape(B, N, NH, HD).transpose(0, 2, 1, 3)

    Q = heads(qf, Wq, bq)
    K = heads(kf, Wk, bk)
    V = heads(kf, Wv, bv)
    attn = np.einsum("bhqd,bhkd->bhqk", Q, K) * (HD ** -0.5)
    attn = attn + float(lambda_p) * P_thermal.reshape(B, 1, N, 1)
    attn = attn - attn.max(axis=-1, keepdims=True)
    w = np.exp(attn)
    w /= w.sum(axis=-1, keepdims=True)
    Vm = V * (1.0 + float(gamma_val) * saliency_map.reshape(B, 1, N, 1))
    Vsp = Vm.transpose(0, 2, 1, 3).reshape(B, N, C).transpose(0, 2, 1)
    Vd = np.empty_like(Vsp)
    for b in range(B):
        GT = _gather_T(offsets[b], np.ones(N, np.float32))
        Vd[b] = Vsp[b] @ GT
    Vdf = Vd.reshape(B, C, N).transpose(0, 2, 1).reshape(B, N, NH, HD).transpose(0, 2, 1, 3)
    out = np.einsum("bhqk,bhkd->bhqd", w, Vdf)
    out = out.transpose(0, 2, 1, 3).reshape(B, N, C)
    out = out @ Wo.T + bo
    return out.transpose(0, 2, 1).reshape(B, C, HH, WW).astype(np.float32)


# --------------------------------------------------------------------------
# device program
# --------------------------------------------------------------------------

def _build_program(chunks):
    """chunks: ordered list of (m, k) gather-tile pairs; same for all cores."""
    nch = len(chunks)
    chunks_for_m = {m: [] for m in range(NKT)}
    for idx, (m, k) in enumerate(chunks):
        chunks_for_m[m].append((idx, k))

    nc = bacc.Bacc(None, target_bir_lowering=False, debug=False)
    xq_d = nc.declare_dram_parameter("xq", [C, NQ], BF16, isOutput=False)
    xkv_d = nc.declare_dram_parameter("xkv", [C, N], BF16, isOutput=False)
    wq_d = nc.declare_dram_parameter("wqT", [C, C], BF16, isOutput=False)
    wk_d = nc.declare_dram_parameter("wkT", [C, C], BF16, isOutput=False)
    wv_d = nc.declare_dram_parameter("wvT", [C, C], BF16, isOutput=False)
    wo_d = nc.declare_dram_parameter("woT", [C, C], BF16, isOutput=False)
    gt_d = nc.declare_dram_parameter("gt", [nch, P, P], BF16, isOutput=False)
    out_d = nc.declare_dram_parameter("outT", [C, NQ], FP32, isOutput=True)

    with tile.TileContext(nc) as tc:
        with tc.tile_pool(name="const", bufs=1) as const, \
             tc.tile_pool(name="work", bufs=1) as work, \
             tc.tile_pool(name="pu_pool", bufs=1) as pu_pool, \
             tc.tile_pool(name="sm", bufs=4) as sm, \
             tc.tile_pool(name="psp", bufs=2, space="PSUM") as psp:

            # ---- per-tile contiguous loads (each DMA = 128 consecutive DRAM
            # rows), spread across the three DMA-capable queues in
            # consumption order: QT needs wq+xq first, then KT needs wk+xkv,
            # then wv / gt / wo.
            def load_rows(engine, dram, nm, nrows_tiles, width, rows_per_tile=P):
                ts = []
                for t in range(nrows_tiles):
                    tl = const.tile([rows_per_tile, width], BF16,
                                    name=f"{nm}{t}", tag=f"{nm}{t}")
                    engine.dma_start(
                        out=tl[:],
                        in_=dram[t * rows_per_tile:(t + 1) * rows_per_tile, :])
                    ts.append(tl)
                return ts

            wq_sb = load_rows(nc.sync, wq_d, "wq", CT, C)
            xq_sb = load_rows(nc.scalar, xq_d, "xq", CT, NQ)
            xkv_sb = load_rows(nc.gpsimd, xkv_d, "xkv", CT, N)
            wk_sb = load_rows(nc.gpsimd, wk_d, "wk", CT, C)
            wv_sb = load_rows(nc.sync, wv_d, "wv", CT, C)
            # gather-matrix chunks and the head-major Wo arrive late in the
            # schedule; single wide DMAs on otherwise-idle queues are fine.
            gt_w = const.tile([P, nch * P], BF16, name="gtw", tag="gtw")
            nc.gpsimd.dma_start(
                out=gt_w[:].rearrange("p (c j) -> p c j", j=P),
                in_=gt_d[:].rearrange("c p j -> p c j"))
            wo_w = const.tile([HD, NH * C], BF16, name="wow", tag="wow")
            nc.sync.dma_start(
                out=wo_w[:].rearrange("p (h n) -> p h n", n=C),
                in_=wo_d[:].rearrange("(h p) n -> p h n", p=HD))

            def wq_s(k, t):
                return wq_sb[k][:, t * P:(t + 1) * P]

            def wk_s(k, t):
                return wk_sb[k][:, t * P:(t + 1) * P]

            def xkv_s(k, lo, width):
                return xkv_sb[k][:, lo: lo + width]

            # ---- emission helpers -----------------------------------------
            qt_sb, kt_sb, v_sb, vd_sb = {}, {}, {}, {}
            pu_tiles, ps_o_tiles = {}, {}
            o_sb = [None] * NH

            def emit_qt(t):
                ps = psp.tile([P, NQ], FP32, name=f"psq{t}", tag="ps_proj", bufs=2)
                for k in range(CT):
                    nc.tensor.matmul(ps[:], lhsT=wq_s(k, t), rhs=xq_sb[k][:],
                                     start=(k == 0), stop=(k == CT - 1))
                qt = work.tile([P, NQ], BF16, name=f"qt{t}", tag=f"qt{t}")
                nc.vector.tensor_copy(qt[:], ps[:])
                qt_sb[t] = qt

            def emit_kt(t):
                kt = work.tile([P, N], BF16, name=f"kt{t}", tag=f"kt{t}")
                for half in range(2):
                    ps = psp.tile([P, NQ], FP32, name=f"psk{t}_{half}",
                                  tag="ps_proj", bufs=2)
                    for k in range(CT):
                        nc.tensor.matmul(ps[:], lhsT=wk_s(k, t),
                                         rhs=xkv_s(k, half * NQ, NQ),
                                         start=(k == 0), stop=(k == CT - 1))
                    nc.vector.tensor_copy(kt[:, half * NQ:(half + 1) * NQ], ps[:])
                kt_sb[t] = kt

            def emit_s_chunk(t, m):
                ps_s = psp.tile([P, 2 * NQ], FP32, name=f"pss{t}_{m}",
                                tag="ps_s", bufs=2)
                # head A on PE rows 0-63, head B on rows 64-127: the two
                # K=64 matmuls occupy disjoint row-groups and overlap.
                kt, qt = kt_sb[t], qt_sb[t]
                nc.tensor.matmul(ps_s[:, 0:NQ],
                                 lhsT=kt[0:HD, m * P:(m + 1) * P],
                                 rhs=qt[0:HD, :], start=True, stop=True)
                nc.tensor.matmul(ps_s[:, NQ:2 * NQ],
                                 lhsT=kt[HD:P, m * P:(m + 1) * P],
                                 rhs=qt[HD:P, :], start=True, stop=True)
                pu = pu_pool.tile([P, 2 * NQ], BF16, name=f"pu{t}_{m}",
                                  tag=f"pu{t}_{m}")
                nc.scalar.activation(out=pu[:], in_=ps_s[:],
                                     func=mybir.ActivationFunctionType.Exp)
                pu_tiles[(t, m)] = pu

            def emit_v(m):
                ps = psp.tile([P, C], FP32, name=f"psv{m}", tag="ps_proj", bufs=2)
                for k in range(CT):
                    nc.tensor.matmul(ps[:], lhsT=xkv_s(k, m * P, P),
                                     rhs=wv_sb[k][:],
                                     start=(k == 0), stop=(k == CT - 1))
                tl = work.tile([P, C], BF16, name=f"v{m}", tag=f"v{m}")
                nc.vector.tensor_copy(tl[:], ps[:])
                v_sb[m] = tl

            def emit_vd(m):
                # grid-sample as banded matmul; vd layout per key-tile:
                # [P, 8*65]; cols h*65..h*65+63 = head h channels,
                # col h*65+64 = 1.0 (softmax denominator trick).
                ps = psp.tile([P, C], FP32, name=f"psvd{m}", tag="ps_proj", bufs=2)
                lst = chunks_for_m[m]
                for j, (idx, k) in enumerate(lst):
                    nc.tensor.matmul(ps[:], lhsT=gt_w[:, idx * P:(idx + 1) * P],
                                     rhs=v_sb[k][:],
                                     start=(j == 0), stop=(j == len(lst) - 1))
                tl = work.tile([P, NH * (HD + 1)], BF16,
                               name=f"vd{m}", tag=f"vd{m}")
                tl3 = tl[:].rearrange("p (h e) -> p h e", e=HD + 1)
                nc.vector.tensor_copy(
                    tl3[:, :, 0:HD],
                    ps[:].rearrange("p (h e) -> p h e", e=HD))
                nc.vector.memset(tl3[:, :, HD:HD + 1], 1.0)
                vd_sb[m] = tl

            def emit_o_chunk(hp, m):
                if hp not in ps_o_tiles:
                    ps_o_tiles[hp] = (
                        psp.tile([P, NQ], FP32, name=f"pso{hp}a", tag="ps_o",
                                 bufs=2),
                        psp.tile([P, NQ], FP32, name=f"pso{hp}b", tag="ps_o",
                                 bufs=2))
                ps_oA, ps_oB = ps_o_tiles[hp]
                hA, hB = 2 * hp, 2 * hp + 1
                pu = pu_tiles[(hp, m)]
                nc.tensor.matmul(
                    ps_oA[0:HD + 1, :],
                    lhsT=vd_sb[m][:, hA * (HD + 1):(hA + 1) * (HD + 1)],
                    rhs=pu[:, 0:NQ],
                    start=(m == 0), stop=(m == NKT - 1))
                nc.tensor.matmul(
                    ps_oB[0:HD + 1, :],
                    lhsT=vd_sb[m][:, hB * (HD + 1):(hB + 1) * (HD + 1)],
                    rhs=pu[:, NQ:2 * NQ],
                    start=(m == 0), stop=(m == NKT - 1))

            # normalization, stage 1 (per pair, right after its O matmuls):
            # evacuate the [65, NQ] PSUM accumulator to SBUF via ACT copy
            # (~0.7us -- frees the ps_o slot fast) and take Ln of the
            # denominator row. By the time the O phase runs, the 32 softmax
            # Exps have drained, so the Ln table set loads exactly once.
            ou_sb = [None] * NH
            ln_sb = [None] * NH

            def emit_evac_ln(hp):
                ps_oA, ps_oB = ps_o_tiles[hp]
                for h, ps_o in ((2 * hp, ps_oA), (2 * hp + 1, ps_oB)):
                    ou = sm.tile([HD + 1, NQ], FP32, name=f"ou{h}", tag=f"ou{h}",
                                 bufs=1)
                    nc.scalar.copy(ou[:], ps_o[0:HD + 1, :])
                    lnr = sm.tile([1, NQ], FP32, name=f"lnr{h}",
                                  tag=f"lnr{h}", bufs=1)
                    nc.scalar.activation(lnr[:], ou[HD:HD + 1, :],
                                         mybir.ActivationFunctionType.Ln)
                    ou_sb[h], ln_sb[h] = ou, lnr

            # normalization, stage 2 (batched at the end): recip = exp(-ln)
            # on ACT (one table switch back to the exp set for all 8 heads),
            # partition-broadcast on GPSIMD, multiply on DVE.
            def emit_recip_mul(h):
                rec = sm.tile([1, NQ], FP32, name=f"rec{h}", tag=f"rec{h}",
                               bufs=1)
                nc.scalar.activation(rec[:], ln_sb[h][:],
                                     mybir.ActivationFunctionType.Exp,
                                     scale=-1.0)
                bc = sm.tile([HD, NQ], FP32, name=f"bc{h}", tag="bc", bufs=4)
                nc.gpsimd.partition_broadcast(bc[:], rec[:])
                ot = work.tile([HD, NQ], BF16, name=f"o{h}", tag=f"o{h}")
                nc.vector.tensor_mul(ot[:], ou_sb[h][0:HD, :], bc[:])
                o_sb[h] = ot

            # ---- emission schedule ----------------------------------------
            for t in range(CT):
                emit_qt(t)
                emit_kt(t)
                for m in range(NKT):
                    emit_s_chunk(t, m)
            for m in range(NKT):
                emit_v(m)
            for m in range(NKT):
                emit_vd(m)
            for hp in range(CT):
                for m in range(NKT):
                    emit_o_chunk(hp, m)
                emit_evac_ln(hp)
            for h in range(NH):
                emit_recip_mul(h)

            # ---- out^T = Wo @ O^T : [C, NQ] fp32 --------------------------
            # h-outer accumulation in two halves: heads 0-5 accumulate while
            # the last head pair is still normalizing, leaving only the h=6,7
            # matmuls in the serial tail. PSUM slots reuse the ps_proj tag
            # (its last user, the Vd accumulators, is long done by now).
            for whalf in range(2):
                tpair = (2 * whalf, 2 * whalf + 1)
                ps_w = {}
                for t in tpair:
                    ps_w[t] = psp.tile([P, NQ], FP32, name=f"psw{t}",
                                       tag="ps_proj", bufs=2)
                for h in range(NH):
                    for t in tpair:
                        nc.tensor.matmul(
                            ps_w[t][:],
                            lhsT=wo_w[:, h * C + t * P: h * C + (t + 1) * P],
                            rhs=o_sb[h][:],
                            start=(h == 0), stop=(h == NH - 1))
                for t in tpair:
                    ob = sm.tile([P, NQ], FP32, name=f"ob{t}", tag="ob", bufs=2)
                    nc.vector.tensor_copy(ob[:], ps_w[t][:])
                    nc.sync.dma_start(out=out_d[t * P:(t + 1) * P, :], in_=ob[:])

    nc.compile()
    return nc


# --------------------------------------------------------------------------
# public entry points
# --------------------------------------------------------------------------

def _prepare(inputs):
    q = np.ascontiguousarray(inputs["q_feat"], np.float32).reshape(B, C, N)
    kv = np.ascontiguousarray(inputs["kv_feat"], np.float32).reshape(B, C, N)
    offsets = np.asarray(inputs["offsets"], np.float32)
    sal = np.asarray(inputs["saliency_map"], np.float32).reshape(B, N)
    gamma = float(np.asarray(inputs["gamma_val"]))

    GTs = [_gather_T(offsets[b], 1.0 + gamma * sal[b]) for b in range(B)]

    # union band-sparsity pattern of the gather matmul across batches, so the
    # SPMD program is identical on every core
    chunks = []
    for m in range(NKT):
        for k in range(NKT):
            if any(GTs[b][k * P:(k + 1) * P, m * P:(m + 1) * P].any()
                   for b in range(B)):
                chunks.append((m, k))

    wqT = np.ascontiguousarray((np.asarray(inputs["Wq"], np.float32).T
                                * (HD ** -0.5)).astype(NP_BF16))
    wkT = np.ascontiguousarray(np.asarray(inputs["Wk"], np.float32).T.astype(NP_BF16))
    wvT = np.ascontiguousarray(np.asarray(inputs["Wv"], np.float32).T.astype(NP_BF16))
    woT = np.ascontiguousarray(np.asarray(inputs["Wo"], np.float32).T.astype(NP_BF16))

    in_maps = []
    for core in range(NCORES):
        b, qh = core // 2, core % 2
        gt_stack = np.stack([
            np.ascontiguousarray(
                GTs[b][k * P:(k + 1) * P, m * P:(m + 1) * P]).astype(NP_BF16)
            for (m, k) in chunks])
        in_maps.append({
            "xq": np.ascontiguousarray(
                q[b][:, qh * NQ:(qh + 1) * NQ]).astype(NP_BF16),
            "xkv": np.ascontiguousarray(kv[b]).astype(NP_BF16),
            "wqT": wqT, "wkT": wkT, "wvT": wvT, "woT": woT,
            "gt": gt_stack,
        })

    def assemble(results):
        out = np.empty((B, C, N), np.float32)
        for core in range(NCORES):
            b, qh = core // 2, core % 2
            out[b][:, qh * NQ:(qh + 1) * NQ] = results[core]["outT"]
        return out.reshape(B, C, HH, WW)

    nc = _build_program(chunks)
    return nc, in_maps, assemble


def _needs_fallback(inputs):
    try:
        if tuple(np.shape(inputs["q_feat"])) != (B, C, HH, WW):
            return True
        for bias in ("bq", "bk", "bv", "bo"):
            if np.any(np.asarray(inputs[bias], np.float32) != 0.0):
                return True
    except Exception:
        return True
    return False


def kernel(**inputs) -> np.ndarray:
    if _needs_fallback(inputs):
        return _reference_numpy(**{k: np.asarray(v, np.float32)
                                   for k, v in inputs.items()})
    nc, in_maps, assemble = _prepare(inputs)
    res = run_bass_kernel_spmd(nc, in_maps, core_ids=list(range(NCORES)))
    return assemble(res.results)


def kernel_traced(trace_cores=(0,), **inputs):
    """Like kernel() but returns (output, exec_time_ns, trace_path)."""
    nc, in_maps, assemble = _prepare(inputs)
    res = run_bass_kernel_spmd(nc, in_maps, core_ids=list(range(NCORES)),
                               trace=True, trace_cores=list(trace_cores))
    trace_path = None
    if res.instructions_and_trace is not None:
        trace_path = res.instructions_and_trace[1]
    return assemble(res.results), res.exec_time_ns, trace_path
